# revision 16
# baseline (speedup 1.0000x reference)
"""Causal self-attention on 8 Trainium2 NeuronCores.

Sharding (matches the batch+head-parallel hint): core c handles batch
b = c // 4 and the 4 heads [hg*4, hg*4+4) where hg = c % 4.  Each core
computes its q/k/v projections from column-sliced c_attn weights, full
causal attention for its heads, and a partial c_proj output from the
matching row slice of w_proj; the host sums the 4 partials per batch.

All matmuls run in float32r (TF32-like rounding, fp32 accumulate).
Causal masking is done on the PE: an identity-weight matmul accumulates
a -1e6 additive mask into the S^T psum tile before the exp.
"""

import sys

if "/opt/trn_rl_repo" not in sys.path:
    sys.path.insert(0, "/opt/trn_rl_repo")

import numpy as np

import concourse.mybir as mybir
from concourse import bacc
from concourse.bass_utils import run_bass_kernel_spmd
from concourse.tile import TileContext

B, T, C = 2, 2048, 1024
H, D = 16, 64
HL = 4  # heads per core
N_CORES = 8
KT = C // 128  # contraction tiles over the embedding dim
SCALE = 1.0 / 8.0  # 1/sqrt(D)
NEG = -1.0e6

_CACHE = {}


def _build():
    f32 = mybir.dt.float32
    f32r = mybir.dt.float32r
    nc = bacc.Bacc("TRN2", target_bir_lowering=False, debug=False, num_devices=N_CORES)

    x_in = nc.dram_tensor("x_in", [128, KT, T], f32r, kind="ExternalInput")
    wqk = nc.dram_tensor("wqk", [128, KT, 2 * HL * D], f32r, kind="ExternalInput")
    wv = nc.dram_tensor("wv", [128, KT, HL * D], f32r, kind="ExternalInput")
    wp = nc.dram_tensor("wp", [128, HL // 2, C], f32r, kind="ExternalInput")
    msk = nc.dram_tensor("msk", [128, 4, 512], f32r, kind="ExternalInput")
    ident = nc.dram_tensor("ident", [128, 128], f32r, kind="ExternalInput")
    out = nc.dram_tensor("out", [T, C], f32, kind="ExternalOutput")

    with TileContext(nc) as tc:
        with tc.tile_pool(name="persist", bufs=1) as persist:
            # q/k feature-major [d, t]: slots 0/1 = q heads {0,1}/{2,3}, 2/3 = k
            qk_t = [persist.tile([128, T], f32r, name=f"qk{s}") for s in range(4)]
            # v token-major per 128-token tile; col D holds ones (denominator)
            v_t = [
                persist.tile([128, HL, D + 1], f32r, name=f"v{tt}") for tt in range(16)
            ]
            # head-pair stacked normalized y per 512-token block
            y2_t = [
                persist.tile([128, HL // 2, 512], f32r, name=f"y2{b_}")
                for b_ in range(4)
            ]
            wp_sb = persist.tile([128, HL // 2, C], f32r)
            msk_sb = persist.tile([128, 4, 512], f32r)
            id_sb = persist.tile([128, 128], f32r)
            nc.sync.dma_start(wp_sb, wp[:, :, :])
            nc.sync.dma_start(msk_sb, msk[:, :, :])
            nc.sync.dma_start(id_sb, ident[:, :])

            ones32 = persist.tile([128, HL, 1], f32)
            nc.vector.memset(ones32, 1.0)
            # ones row at partition D for the K=1 denominator broadcast
            onesbc32 = persist.tile([D + 1, D], f32)
            nc.vector.memset(onesbc32[D : D + 1, :], 1.0)
            onesbc = persist.tile([D + 1, D], f32r)
            nc.vector.tensor_copy(onesbc[D : D + 1, :], onesbc32[D : D + 1, :])

            with (
                tc.tile_pool(name="qkvp", bufs=1) as qkvp,
                tc.tile_pool(name="ps_qk", bufs=3, space="PSUM") as ps_qk,
                tc.tile_pool(name="ps_v", bufs=3, space="PSUM") as ps_v,
            ):
                x_sb = qkvp.tile([128, KT, T], f32r)
                wqk_sb = qkvp.tile([128, KT, 2 * HL * D], f32r)
                wv_sb = qkvp.tile([128, KT, HL * D], f32r)
                nc.sync.dma_start(wqk_sb, wqk[:, :, :])
                nc.sync.dma_start(wv_sb, wv[:, :, :])
                for kt in range(KT):
                    nc.sync.dma_start(x_sb[:, kt, :], x_in[:, kt, :])

                for jt in range(4):
                    for tb in range(4):
                        qk_ps = ps_qk.tile([128, 512], f32)
                        for kt in range(KT):
                            nc.tensor.matmul(
                                qk_ps,
                                wqk_sb[:, kt, jt * 128 : (jt + 1) * 128],
                                x_sb[:, kt, tb * 512 : (tb + 1) * 512],
                                start=(kt == 0),
                                stop=(kt == KT - 1),
                            )
                        nc.vector.tensor_copy(
                            qk_t[jt][:, tb * 512 : (tb + 1) * 512], qk_ps
                        )
                for tt in range(16):
                    v_ps = ps_v.tile([128, HL * D], f32)
                    for kt in range(KT):
                        nc.tensor.matmul(
                            v_ps,
                            x_sb[:, kt, tt * 128 : (tt + 1) * 128],
                            wv_sb[:, kt, :],
                            start=(kt == 0),
                            stop=(kt == KT - 1),
                        )
                    nc.vector.tensor_copy(
                        v_t[tt][:, :, 0:D],
                        v_ps.rearrange("p (h d) -> p h d", h=HL),
                    )
                    nc.vector.tensor_copy(v_t[tt][:, :, D : D + 1], ones32)

            with (
                tc.tile_pool(name="attp", bufs=3) as attp,
                tc.tile_pool(name="attsmall", bufs=4) as attsmall,
                tc.tile_pool(name="projp", bufs=3) as projp,
                tc.tile_pool(name="ps_st", bufs=2, space="PSUM") as ps_st,
                tc.tile_pool(name="ps_y", bufs=2, space="PSUM") as ps_y,
                tc.tile_pool(name="ps_rb", bufs=1, space="PSUM") as ps_rb,
                tc.tile_pool(name="ps_o", bufs=1, space="PSUM") as ps_o,
            ):
                for jq in range(4):
                    for h in range(HL):
                        qslot = h // 2
                        kslot = 2 + h // 2
                        base = (h % 2) * D
                        pr = h // 2
                        y_ps = ps_y.tile([D + 1, 512], f32, name="y_ps")
                        njt = 4 * (jq + 1)
                        npair = njt // 2

                        def s_pair(p):
                            st = ps_st.tile([128, 2, 512], f32, name="st")
                            for s in range(2):
                                j = 2 * p + s
                                diag = j >= 4 * jq
                                w = max(0, (j - 4 * jq) * 128)
                                nc.tensor.matmul(
                                    st[:, s, w:],
                                    qk_t[kslot][
                                        base : base + D, j * 128 : (j + 1) * 128
                                    ],
                                    qk_t[qslot][
                                        base : base + D,
                                        jq * 512 + w : (jq + 1) * 512,
                                    ],
                                    start=True,
                                    stop=not diag,
                                )
                                if diag:
                                    # accumulate -1e6 additive causal mask
                                    nc.tensor.matmul(
                                        st[:, s, :],
                                        id_sb,
                                        msk_sb[:, j - 4 * jq, :],
                                        start=False,
                                        stop=True,
                                    )
                            est = attp.tile([128, 2, 512], f32r, tag="est", name="est")
                            nc.scalar.activation(
                                est, st, mybir.ActivationFunctionType.Exp, scale=SCALE
                            )
                            return est

                        def pv_pair(p, est):
                            for s in range(2):
                                j = 2 * p + s
                                nc.tensor.matmul(
                                    y_ps,
                                    v_t[j][:, h, :],
                                    est[:, s, :],
                                    start=(j == 0),
                                    stop=(j == njt - 1),
                                )

                        prev = None
                        for p in range(npair):
                            cur = s_pair(p)
                            if prev is not None:
                                pv_pair(p - 1, prev)
                            prev = cur
                        pv_pair(npair - 1, prev)

                        # normalize: row D of y_ps is the softmax denominator
                        r_sb = attsmall.tile([D + 1, 512], f32r, tag="rr")
                        nc.scalar.copy(r_sb[D : D + 1, :], y_ps[D : D + 1, :])
                        rb_ps = ps_rb.tile([D, 512], f32)
                        nc.tensor.matmul(
                            rb_ps,
                            onesbc[D : D + 1, :],
                            r_sb[D : D + 1, :],
                            start=True,
                            stop=True,
                        )
                        rb_sb = attsmall.tile([D, 512], f32, tag="rb")
                        nc.vector.reciprocal(rb_sb, rb_ps)
                        if h % 2 == 0:
                            nc.vector.tensor_mul(
                                y2_t[jq][0:D, pr, :], y_ps[0:D, :], rb_sb
                            )
                        else:
                            y_lo = attsmall.tile([D, 512], f32r, tag="ylo")
                            nc.vector.tensor_mul(y_lo, y_ps[0:D, :], rb_sb)
                            nc.gpsimd.dma_start(y2_t[jq][D:128, pr, :], y_lo)

                # output projection (partial over this core's heads)
                npr = HL // 2
                for tt in range(16):
                    o_sb = projp.tile([128, C], f32)
                    blk, off = tt // 4, (tt % 4) * 128
                    for cb in range(2):
                        o_ps = ps_o.tile([128, 512], f32)
                        for pr in range(npr):
                            nc.tensor.matmul(
                                o_ps,
                                y2_t[blk][:, pr, off : off + 128],
                                wp_sb[:, pr, cb * 512 : (cb + 1) * 512],
                                start=(pr == 0),
                                stop=(pr == npr - 1),
                            )
                        nc.vector.tensor_copy(o_sb[:, cb * 512 : (cb + 1) * 512], o_ps)
                    nc.sync.dma_start(out[tt * 128 : (tt + 1) * 128, :], o_sb)

    nc.compile()
    return nc


def _get_nc():
    if "nc" not in _CACHE:
        _CACHE["nc"] = _build()
    return _CACHE["nc"]


def make_in_maps(x, w_attn, w_proj):
    x = np.asarray(x, np.float32)
    w_attn = np.asarray(w_attn, np.float32)
    w_proj = np.asarray(w_proj, np.float32)

    # additive causal masks for the 4 diagonal-straddling block offsets:
    # msk[p, wi, y] = 0 where kept (y >= wi*128 + p), else -1e6
    p_ = np.arange(128)[:, None]
    y_ = np.arange(512)[None, :]
    msk = np.zeros((128, 4, 512), np.float32)
    for wi in range(4):
        msk[:, wi, :] = np.where(y_ >= wi * 128 + p_, 0.0, NEG)
    ident = np.eye(128, dtype=np.float32)

    in_maps = []
    for c in range(N_CORES):
        b, hg = c // 4, c % 4
        hs = hg * HL * D  # 256 * hg
        xt = np.ascontiguousarray(x[b].T)  # [C, T]
        x_t = xt.reshape(KT, 128, T).transpose(1, 0, 2)
        wq = w_attn[hs : hs + HL * D, :]
        wk = w_attn[C + hs : C + hs + HL * D, :]
        wqkt = np.concatenate([wq, wk], 0).T  # [C, 512]
        wqk_t = wqkt.reshape(KT, 128, 2 * HL * D).transpose(1, 0, 2)
        wvt = w_attn[2 * C + hs : 2 * C + hs + HL * D, :].T  # [C, 256]
        wv_t = wvt.reshape(KT, 128, HL * D).transpose(1, 0, 2)
        # head-pair stacked rows: [128, HL//2, C]; partition p of pair pr is
        # local feature pr*128 + p (head 2*pr dims then head 2*pr+1 dims)
        wp_t = (
            w_proj[:, hs : hs + HL * D].T.reshape(HL // 2, 128, C).transpose(1, 0, 2)
        )
        in_maps.append(
            {
                "x_in": np.ascontiguousarray(x_t, np.float32),
                "wqk": np.ascontiguousarray(wqk_t, np.float32),
                "wv": np.ascontiguousarray(wv_t, np.float32),
                "wp": np.ascontiguousarray(wp_t, np.float32),
                "msk": msk,
                "ident": ident,
            }
        )
    return in_maps


def run(in_maps, **kwargs):
    nc = _get_nc()
    return run_bass_kernel_spmd(nc, in_maps, core_ids=list(range(N_CORES)), **kwargs)


def combine(results):
    out = np.zeros((B, T, C), np.float64)
    for c in range(N_CORES):
        out[c // 4] += results[c]["out"].astype(np.float64)
    return out.astype(np.float32)


def kernel(x, w_attn, w_proj):
    res = run(make_in_maps(x, w_attn, w_proj))
    return combine(res.results)
